# revision 4
# baseline (speedup 1.0000x reference)
"""Bass/Trainium2 kernel for nn_KernelizedAttentionResBlock (v6).

Per core (n-token sharded: 128 rows of n, all 32 batches):
  stream:  per batch b: St = DerivErf(K*s_b + t_b) on Act (fused scale+bias),
           xT[:,b] = 0.8862*sum_d(St*V) + Q^T[:,b] via one DVE
           tensor_tensor_reduce. fp16 K/V, 4-batch DMA blocks; the last
           block streams at 2-batch granularity (K first) so the drain is
           short. cc_in staging DMAs are emitted BEFORE the next K/V
           prefetch so they preempt it in the DMA-engine FIFO.
  gather:  two AllGathers split by batch (A = b0..15 hidden under the
           stream; B = b16..31 issued at stream end). Rows 128..129 of the
           payload carry partial [sum(x)/128, sum(x^2)/128] so LN stats are
           3 pairwise adds post-gather.
  ffn:     m-sharded. h1raw = w1f @ x starts as soon as gathered x lands;
           LN applied post-matmul: z = h1raw*r - (r*m)*w1s + b1 with
           r = rsqrt(var) computed ON DVE via the bit-trick seed + Newton
           steps (no act-table switch), sigma = Sigmoid(z+b1) on Act (the
           only post-stream Act function -> zero table loads on the tail),
           g = (z+b1)*sigma. FFN for half A runs inside AG-B's collective
           window; only half B's FFN is exposed.
  weights: W1T/B1/W1S/W2T DMAs gated on the last cc_in staging DMA so they
           never delay the stream or the AG-B dispatch.
Host combines: out = x + sum_c HP_c + b2.
"""
import sys
import os

sys.path.insert(0, "/opt/trn_rl_repo")

import numpy as np

N = 1024
B = 32
D = 1024
M = 4096
NCORES = 8
NSL = N // NCORES
MSL = M // NCORES
MCH = MSL // 128
LN_EPS = 1e-5
NB = 4
NBLK = B // NB
BA = 16               # batches in gather half A
BB = B - BA
ERF_SCALE = 0.8862269254527579  # sqrt(pi)/2
RSQRT_MAGIC = float(np.frombuffer(np.uint32(0x5F3759DF).tobytes(),
                                  dtype=np.float32)[0])

_built = {}
last_results = None


def _build_module():
    if "nc" in _built:
        return _built["nc"]

    import concourse.bacc as bacc
    import concourse.mybir as mybir
    import concourse.tile as tile

    AF = mybir.ActivationFunctionType
    ALU = mybir.AluOpType
    f32 = mybir.dt.float32
    f16 = mybir.dt.float16
    i32 = mybir.dt.int32

    nc = bacc.Bacc(trn_type="TRN2", num_devices=NCORES)

    Kd = nc.dram_tensor("Ks", (NSL, B, D), f16, kind="ExternalInput")
    Vd = nc.dram_tensor("Vs", (NSL, B, D), f16, kind="ExternalInput")
    STQ = nc.dram_tensor("STQ", (NSL, 3, B), f32, kind="ExternalInput")
    W1T = nc.dram_tensor("W1T", (N, MSL), f16, kind="ExternalInput")
    B1 = nc.dram_tensor("B1", (128, MCH), f32, kind="ExternalInput")
    W1S = nc.dram_tensor("W1S", (128, MCH), f32, kind="ExternalInput")
    W2T = nc.dram_tensor("W2T", (MSL, N), f16, kind="ExternalInput")

    XTd = nc.dram_tensor("XT", (NSL, B), f32, kind="ExternalOutput")
    HPd = nc.dram_tensor("HP", (N, B), f32, kind="ExternalOutput")

    ccA_in = nc.dram_tensor("ccA_in", (NSL + 2, BA), f16, kind="Internal")
    ccA_out = nc.dram_tensor("ccA_out", (NCORES * (NSL + 2), BA), f16,
                             kind="Internal", addr_space="Shared")
    ccB_in = nc.dram_tensor("ccB_in", (NSL + 2, BB), f16, kind="Internal")
    ccB_out = nc.dram_tensor("ccB_out", (NCORES * (NSL + 2), BB), f16,
                             kind="Internal", addr_space="Shared")

    with tile.TileContext(nc) as tc:
        with tc.tile_pool(name="const", bufs=1) as cst, \
             tc.tile_pool(name="small", bufs=1) as sm, \
             tc.tile_pool(name="kp", bufs=3) as kp, \
             tc.tile_pool(name="vp", bufs=3) as vp, \
             tc.tile_pool(name="st", bufs=4) as stp, \
             tc.tile_pool(name="pr", bufs=2) as prp, \
             tc.tile_pool(name="psum", bufs=1, space="PSUM") as ps:

            stq = cst.tile([NSL, 3, B], f32)
            nc.sync.dma_start(stq[:], STQ[:])

            gate = nc.alloc_semaphore("wgate")

            xT = sm.tile([NSL, B], f32)
            xT16 = sm.tile([NSL, B], f16)
            xsq = sm.tile([NSL, B], f16)
            ones_h = cst.tile([128, 1], f16)
            nc.vector.memset(ones_h[:], 1.0 / 128.0)
            ones_r = cst.tile([1, 128], f32)
            nc.vector.memset(ones_r[:], 1.0)
            magic = cst.tile([1, BA], f32)
            nc.vector.memset(magic[:], RSQRT_MAGIC)

            kts = {}
            vts = {}

            def _load_kv(blk, cc_mid=None):
                b0 = blk * NB
                kt = kp.tile([NSL, NB, D], f16, tag="kt", name=f"kt{blk}")
                vt = vp.tile([NSL, NB, D], f16, tag="vt", name=f"vt{blk}")
                kts[blk], vts[blk] = kt, vt
                if blk == 0:
                    nc.sync.dma_start(kt[:, 0:2, :], Kd[:, 0:2, :])
                    nc.sync.dma_start(vt[:, 0:2, :], Vd[:, 0:2, :])
                    nc.sync.dma_start(kt[:, 2:4, :], Kd[:, 2:4, :])
                    nc.sync.dma_start(vt[:, 2:4, :], Vd[:, 2:4, :])
                elif blk >= NBLK - 2:
                    # tail blocks: K first (acts lead ttrs), V in half-D
                    # pieces so the ttrs pipeline with the V arrival
                    nc.sync.dma_start(kt[:, 0:2, :], Kd[:, b0:b0 + 2, :])
                    nc.sync.dma_start(kt[:, 2:4, :], Kd[:, b0 + 2:b0 + 4, :])
                    for i in range(NB):
                        for h in range(2):
                            d0, d1 = h * (D // 2), (h + 1) * (D // 2)
                            nc.sync.dma_start(vt[:, i:i + 1, d0:d1],
                                              Vd[:, b0 + i:b0 + i + 1, d0:d1])
                else:
                    nc.sync.dma_start(kt[:], Kd[:, b0:b0 + NB, :])
                    nc.sync.dma_start(vt[:], Vd[:, b0:b0 + NB, :])

            def _act(b):
                blk, i = divmod(b, NB)
                st = stp.tile([NSL, D], f16, tag="st", name=f"st{b % 4}")
                nc.scalar.activation(st[:], kts[blk][:, i, :],
                                     AF.Derivative_Erf,
                                     bias=stq[:, 1, b:b + 1],
                                     scale=stq[:, 0, b:b + 1])
                return st

            def _batch(b, split=False):
                blk, i = divmod(b, NB)
                st = _act(b)
                prod = prp.tile([NSL, D], f16, tag="pr", name=f"pr{b % 2}")
                if not split:
                    nc.vector.tensor_tensor_reduce(
                        prod[:], st[:], vts[blk][:, i, :], ERF_SCALE,
                        stq[:, 2, b:b + 1], ALU.mult, ALU.add, xT[:, b:b + 1])
                else:
                    h = D // 2
                    nc.vector.tensor_tensor_reduce(
                        prod[:, 0:h], st[:, 0:h], vts[blk][:, i, 0:h],
                        ERF_SCALE, stq[:, 2, b:b + 1], ALU.mult, ALU.add,
                        xT[:, b:b + 1])
                    nc.vector.tensor_tensor_reduce(
                        prod[:, h:D], st[:, h:D], vts[blk][:, i, h:D],
                        ERF_SCALE, xT[:, b:b + 1], ALU.mult, ALU.add,
                        xT[:, b:b + 1])
                nc.vector.tensor_copy(xT16[:, b:b + 1], xT[:, b:b + 1])
                if b >= BA:
                    nc.vector.tensor_tensor(xsq[:, b:b + 1], xT16[:, b:b + 1],
                                            xT16[:, b:b + 1], op=ALU.mult)

            def _batch_pool(b):
                """Fold-chain reduce on the (otherwise idle) Pool engine."""
                blk, i = divmod(b, NB)
                st = _act(b)
                gp = nc.gpsimd
                sv = sm.tile([NSL, D], f16, name=f"sv{b}")
                gp.tensor_tensor(sv[:], st[:], vts[blk][:, i, :], op=ALU.mult)
                f1 = sm.tile([NSL, D // 2], f16, name=f"f1{b}")
                gp.tensor_tensor(f1[:], sv[:, 0:512], sv[:, 512:1024],
                                 op=ALU.add)
                f2 = sm.tile([NSL, D // 4], f16, name=f"f2{b}")
                gp.tensor_tensor(f2[:], f1[:, 0:256], f1[:, 256:512],
                                 op=ALU.add)
                f3 = sm.tile([NSL, D // 8], f16, name=f"f3{b}")
                gp.tensor_tensor(f3[:], f2[:, 0:128], f2[:, 128:256],
                                 op=ALU.add)
                f4 = sm.tile([NSL, D // 16], f16, name=f"f4{b}")
                gp.tensor_tensor(f4[:], f3[:, 0:64], f3[:, 64:128],
                                 op=ALU.add)
                red = sm.tile([NSL, 1], f32, name=f"red{b}")
                nc.vector.tensor_reduce(red[:], f4[:], op=ALU.add,
                                        axis=mybir.AxisListType.X)
                nc.vector.tensor_scalar(xT[:, b:b + 1], red[:], ERF_SCALE,
                                        stq[:, 2, b:b + 1], op0=ALU.mult,
                                        op1=ALU.add)
                gp.tensor_copy(xT16[:, b:b + 1], xT[:, b:b + 1])
                if b >= BA:
                    gp.tensor_tensor(xsq[:, b:b + 1], xT16[:, b:b + 1],
                                     xT16[:, b:b + 1], op=ALU.mult)

            def _stats(lo, hi, t0, t1, copy_eng):
                w = hi - lo
                if lo == 0:
                    nc.vector.tensor_tensor(xsq[:, lo:hi], xT16[:, lo:hi],
                                            xT16[:, lo:hi], op=ALU.mult)
                mm = ps.tile([1, w], f32, tag=t0)
                nc.tensor.matmul(mm[:], ones_h[:], xT16[:, lo:hi],
                                 start=True, stop=True)
                qq = ps.tile([1, w], f32, tag=t1)
                nc.tensor.matmul(qq[:], ones_h[:], xsq[:, lo:hi],
                                 start=True, stop=True)
                stat = sm.tile([1, 2 * w], f16, name=f"stat{lo}")
                copy_eng(stat[:, 0:w], mm[:])
                copy_eng(stat[:, w:2 * w], qq[:])
                return stat

            # ---------- stream ----------
            # emission per block: batches, K/V prefetch, then cc staging
            # DMAs (which SP-block until their data is computed, slotting
            # their transfers right behind the prefetched block)
            _load_kv(0)
            _load_kv(1)
            for blk in range(NBLK):
                last = blk == NBLK - 1
                for i in range(NB):
                    b = blk * NB + i
                    if i == 2 and b < 8:
                        _batch_pool(b)
                    else:
                        _batch(b, split=(blk >= NBLK - 2))
                if blk + 2 < NBLK:
                    _load_kv(blk + 2)
                if blk == 1:
                    nc.sync.dma_start(ccA_in[0:NSL, 0:8], xT16[:, 0:8])
                if blk == 2:
                    nc.sync.dma_start(ccA_in[0:NSL, 8:12], xT16[:, 8:12])
                if blk == 3:
                    nc.sync.dma_start(ccA_in[0:NSL, 12:BA], xT16[:, 12:BA])
                    statA = _stats(0, BA, "p0", "p1", nc.vector.tensor_copy)
                    nc.sync.dma_start(ccA_in[NSL:NSL + 2, :].rearrange("a b -> (a b)"), statA[:])
                    nc.gpsimd.collective_compute(
                        "AllGather", mybir.AluOpType.bypass,
                        replica_groups=[list(range(NCORES))],
                        ins=[ccA_in[:]], outs=[ccA_out[:]],
                    )
                if blk == 5:
                    nc.sync.dma_start(ccB_in[0:NSL, 0:8], xT16[:, BA:24])
                if blk == 6:
                    nc.sync.dma_start(ccB_in[0:NSL, 8:12], xT16[:, 24:28])

            # drain: b28..31 staged + stats B + AG-B
            ccb2 = nc.sync.dma_start(ccB_in[0:NSL, 12:BB], xT16[:, 28:B])
            statB = _stats(BA, B, "p2", "p3", nc.scalar.copy)
            sbd = nc.sync.dma_start(ccB_in[NSL:NSL + 2, :].rearrange("a b -> (a b)"), statB[:])
            sbd.then_inc(gate, 16)
            nc.gpsimd.collective_compute(
                "AllGather", mybir.AluOpType.bypass,
                replica_groups=[list(range(NCORES))],
                ins=[ccB_in[:]], outs=[ccB_out[:]],
            )

            # gathered-A loads first (DMA is idle right after the stream),
            # then weights (gated behind the drain staging DMAs)
            pstatA_t = sm.tile([1, 2, BA, NCORES], f16, name="pstatA")
            nc.sync.dma_start(
                pstatA_t[:],
                ccA_out[:].rearrange("(c j) b -> j b c", j=NSL + 2)
                [128:130].unsqueeze(0))
            xThA_t = sm.tile([128, NCORES, BA], f16, name="xThA")
            nc.sync.dma_start(
                xThA_t[:],
                ccA_out[:].rearrange("(c j) b -> j c b", j=NSL + 2)[0:128])
            w1T = cst.tile([128, NCORES, MSL], f16)
            nc.sync.dma_start(
                w1T[:], W1T[:].rearrange("(c p) m -> p c m", p=128)
            )._wait_ge(gate, 16)
            b1 = cst.tile([128, MCH], f32)
            nc.sync.dma_start(b1[:], B1[:])
            w1s = cst.tile([128, MCH], f32)
            nc.sync.dma_start(w1s[:], W1S[:])
            w2T = cst.tile([128, MCH, N], f16)
            nc.sync.dma_start(w2T[:], W2T[:].rearrange("(mi p) n -> p mi n", p=128))
            nc.sync.dma_start(XTd[:], xT[:])

            # ---------- FFN halves ----------
            def _ffn_half(cc_out, W, HPslice, tag, nr_iters, pstat, xTh,
                          h1_first):
                if pstat is None:
                    pstat = sm.tile([1, 2, W, NCORES], f16, name=f"pstat{tag}")
                    nc.sync.dma_start(
                        pstat[:],
                        cc_out[:].rearrange("(c j) b -> j b c",
                                            j=NSL + 2)[128:130].unsqueeze(0))
                    xTh = sm.tile([128, NCORES, W], f16, name=f"xTh{tag}")
                    nc.sync.dma_start(
                        xTh[:],
                        cc_out[:].rearrange("(c j) b -> j c b",
                                            j=NSL + 2)[0:128])

                # h1raw = w1f @ x on raw gathered x (no stats dependency)
                h1raws = []

                def _h1raw():
                    for mi in range(MCH):
                        h1t = ps.tile([128, W], f32, tag=("p4", "p5")[mi % 2],
                                      name=f"h1{tag}{mi}")
                        for c in range(NCORES):
                            nc.tensor.matmul(
                                h1t[:], w1T[:, c, mi * 128:(mi + 1) * 128],
                                xTh[:, c, :],
                                start=(c == 0), stop=(c == NCORES - 1))
                        h1raws.append(h1t)

                if h1_first:
                    _h1raw()

                # stats: sum partials, var, r = rsqrt(var) all on DVE
                s1 = sm.tile([1, 2, W], f32, name=f"s1{tag}")
                nc.vector.tensor_reduce(
                    s1[:], pstat[:],
                    op=ALU.add, axis=mybir.AxisListType.X)
                mrow = sm.tile([1, W], f32, name=f"m{tag}")
                nc.vector.tensor_scalar_mul(mrow[:], s1[:, 0, :], 0.125)
                u = sm.tile([1, W], f32, name=f"u{tag}")
                nc.vector.tensor_tensor(u[:], mrow[:], mrow[:], op=ALU.mult)
                var = sm.tile([1, W], f32, name=f"v{tag}")
                nc.vector.scalar_tensor_tensor(var[:], s1[:, 1, :], 0.125,
                                               u[:], op0=ALU.mult,
                                               op1=ALU.subtract)
                # r = rsqrt(var): magic seed + Newton steps (pure DVE)
                sh = sm.tile([1, W], f32, name=f"sh{tag}")
                nc.vector.tensor_scalar(sh[:].bitcast(i32),
                                        var[:].bitcast(i32), 1, None,
                                        op0=ALU.arith_shift_right)
                y = sm.tile([1, W], f32, name=f"y{tag}")
                nc.vector.tensor_tensor(y[:].bitcast(i32),
                                        magic[:, 0:W].bitcast(i32),
                                        sh[:].bitcast(i32), op=ALU.subtract)
                for it in range(nr_iters):
                    a = sm.tile([1, W], f32, name=f"a{tag}{it}")
                    nc.vector.tensor_tensor(a[:], y[:], y[:], op=ALU.mult)
                    bb = sm.tile([1, W], f32, name=f"b{tag}{it}")
                    nc.vector.tensor_tensor(bb[:], var[:], a[:], op=ALU.mult)
                    hh = sm.tile([1, W], f32, name=f"h{tag}{it}")
                    nc.vector.tensor_scalar(hh[:], bb[:], -0.5, 1.5,
                                            op0=ALU.mult, op1=ALU.add)
                    y2 = sm.tile([1, W], f32, name=f"y2{tag}{it}")
                    nc.vector.tensor_tensor(y2[:], y[:], hh[:], op=ALU.mult)
                    y = y2
                rmrow = sm.tile([1, W], f32, name=f"rm{tag}")
                nc.vector.tensor_tensor(rmrow[:], y[:], mrow[:], op=ALU.mult)

                R1p = ps.tile([128, W], f32, tag="p6", name=f"R{tag}")
                nc.tensor.matmul(R1p[:], ones_r[:], y[:], start=True, stop=True)
                RM1 = ps.tile([128, W], f32, tag="p7", name=f"RM{tag}")
                nc.tensor.matmul(RM1[:], ones_r[:], rmrow[:],
                                 start=True, stop=True)
                R1 = sm.tile([128, W], f32, name=f"R1s{tag}")
                nc.vector.tensor_copy(R1[:], R1p[:])
                if not h1_first:
                    _h1raw()

                # zneg = (r*m)*w1s - r*h1raw = -(z - b1-part); sigma =
                # Sigmoid(-zneg + b1); gneg = (zneg - b1)*sigma = -g.
                # Host negates w2 so h2 comes out with the right sign.
                g1 = sm.tile([128, MCH, W], f16, name=f"g1{tag}")
                for mi in range(MCH):
                    t1 = sm.tile([128, W], f32, tag=f"t1{mi % 2}")
                    nc.vector.tensor_tensor(t1[:], h1raws[mi][:], R1[:],
                                            op=ALU.mult)
                    zn = sm.tile([128, W], f32, tag=f"z{mi % 2}")
                    nc.vector.scalar_tensor_tensor(
                        zn[:], RM1[:], w1s[:, mi:mi + 1], t1[:],
                        op0=ALU.mult, op1=ALU.subtract)
                    sg = sm.tile([128, W], f32, tag=f"sg{mi % 2}")
                    nc.scalar.activation(sg[:], zn[:], AF.Sigmoid,
                                         scale=-1.0, bias=b1[:, mi:mi + 1])
                    nc.vector.scalar_tensor_tensor(
                        g1[:, mi, :], zn[:], b1[:, mi:mi + 1], sg[:],
                        op0=ALU.subtract, op1=ALU.mult)

                hp_sb = sm.tile([128, NCORES, W], f32, name=f"hp{tag}")
                for jn in range(NCORES):
                    hpt = ps.tile([128, W], f32,
                                  tag=("p0", "p1", "p2", "p3")[jn % 4])
                    for mi in range(MCH):
                        nc.tensor.matmul(hpt[:],
                                         w2T[:, mi, jn * 128:(jn + 1) * 128],
                                         g1[:, mi, :],
                                         start=(mi == 0), stop=(mi == MCH - 1))
                    if jn % 2 == 0:
                        nc.vector.tensor_copy(hp_sb[:, jn, :], hpt[:])
                    else:
                        nc.scalar.copy(hp_sb[:, jn, :], hpt[:])
                nc.sync.dma_start(
                    HPd[:].rearrange("(jn p) b -> p jn b", p=128)
                    [:, :, HPslice], hp_sb[:])

            _ffn_half(ccA_out, BA, slice(0, BA), "A", 2,
                      pstatA_t, xThA_t, h1_first=False)
            _ffn_half(ccB_out, BB, slice(BA, B), "B", 1,
                      None, None, h1_first=True)

    nc.finalize()
    _built["nc"] = nc
    return nc


def kernel(**inputs):
    from concourse.bass_utils import run_bass_kernel_spmd

    global last_results

    Q = np.asarray(inputs["Q"], dtype=np.float32)
    K = np.asarray(inputs["K"], dtype=np.float32)
    V = np.asarray(inputs["V"], dtype=np.float32)
    mu_w = np.asarray(inputs["mu_w"], dtype=np.float32)
    mu_b = np.asarray(inputs["mu_b"], dtype=np.float32)
    sigma_w = np.asarray(inputs["sigma_w"], dtype=np.float32)
    sigma_b = np.asarray(inputs["sigma_b"], dtype=np.float32)
    ffn_w1 = np.asarray(inputs["ffn_w1"], dtype=np.float32)
    ffn_b1 = np.asarray(inputs["ffn_b1"], dtype=np.float32)
    ffn_w2 = np.asarray(inputs["ffn_w2"], dtype=np.float32)
    ffn_b2 = np.asarray(inputs["ffn_b2"], dtype=np.float32)
    ln_ff_g = np.asarray(inputs["ln_ff_g"], dtype=np.float32)
    ln_ff_b = np.asarray(inputs["ln_ff_b"], dtype=np.float32)
    ln_q_g = np.asarray(inputs["ln_q_g"], dtype=np.float32)
    ln_q_b = np.asarray(inputs["ln_q_b"], dtype=np.float32)

    qmu = Q.mean(axis=-1, keepdims=True)
    qvar = Q.var(axis=-1, keepdims=True)
    qn = (Q - qmu) / np.sqrt(qvar + LN_EPS) * ln_q_g + ln_q_b
    mu = np.tanh(qn @ mu_w.T + mu_b)
    sg = qn @ sigma_w.T + sigma_b
    sfull = 1.0 / np.sqrt(2.0 * (sg * sg + 1e-8))
    tfull = -sfull * mu

    w1f = ffn_w1 * ln_ff_g[None, :]
    b1f = ffn_b1 + ffn_w1 @ ln_ff_b
    w1sums = w1f.sum(axis=1)

    STQ_full = np.ascontiguousarray(
        np.stack([sfull.T, tfull.T, Q.T], axis=1)).astype(np.float32)
    w1T = np.ascontiguousarray(w1f.T).astype(np.float16)
    # device computes g1 NEGATED (sign-flip epilogue) -> negate w2 here
    w2T = np.ascontiguousarray(-ffn_w2.T).astype(np.float16)
    K16 = K.astype(np.float16)
    V16 = V.astype(np.float16)

    nc = _build_module()

    in_maps = []
    for c in range(NCORES):
        jsl = slice(c * NSL, (c + 1) * NSL)
        msl = slice(c * MSL, (c + 1) * MSL)
        in_maps.append({
            "Ks": np.ascontiguousarray(K16[:, jsl, :].transpose(1, 0, 2)),
            "Vs": np.ascontiguousarray(V16[:, jsl, :].transpose(1, 0, 2)),
            "STQ": np.ascontiguousarray(STQ_full[jsl]),
            "W1T": np.ascontiguousarray(w1T[:, msl]),
            "B1": np.ascontiguousarray(b1f[msl].reshape(MCH, 128).T),
            "W1S": np.ascontiguousarray(w1sums[msl].reshape(MCH, 128).T),
            "W2T": np.ascontiguousarray(w2T[msl, :]),
        })

    trace = os.environ.get("BASS_KERNEL_TRACE", "0") == "1"
    res = run_bass_kernel_spmd(
        nc, in_maps, core_ids=list(range(NCORES)), trace=trace
    )
    last_results = res

    x = np.concatenate([res.results[c]["XT"] for c in range(NCORES)], axis=0).T
    h = np.zeros((N, B), dtype=np.float32)
    for c in range(NCORES):
        h += res.results[c]["HP"]
    out = x + h.T + ffn_b2[None, :]
    return out.astype(np.float32)
